# revision 1
# baseline (speedup 1.0000x reference)
"""Trainium2 Bass kernel for the autoregressive LSTM decoder (nn_Decoder).

Problem: single-layer LSTM (H=1024) decoder, B=128 examples, T=256 steps,
autoregressive scalar feedback through a linear head (OUT=1), extra flag
input. Sequential over T.

Strategy (data-parallel, zero communication):
  * Shard batch over 8 cores (16 examples/core); replicate weights.
  * Per step compute gates in TRANSPOSED layout (gate-dim on partitions,
    batch on free dim) so the LSTM cell elementwise runs on all 128 lanes:
        gates.T = A @ h_ext
    A is a (4224 x 1028) fused matrix built on host:
      - W_eff = W_hh + outer(W_ih[:,0], W_lin): the scalar feedback
        out(t-1) = W_lin h(t-1) + b_lin is folded into the recurrent
        matrix (rank-1 update), eliminating the serial feedback path.
      - extra columns carry the fused bias and step-0 corrections, driven
        by constant rows appended to h_ext (so no per-tile bias adds).
      - row-tile 32 carries W_lin: the same matmul emits out(t-1).
  * Weights held in SBUF as fp16 (PE fast-weight-load), state h fp16,
    cell state c and all accumulation/elementwise in fp32.
"""

import os
import numpy as np
import ml_dtypes

import concourse.bass as bass
import concourse.tile as tile
import concourse.bass_utils as bass_utils
from concourse import bacc, mybir

H = 1024
B = 128
T = 256
N_CORES = 8
BC = B // N_CORES          # batch per core = 16
MT = 33                     # row tiles of A (32 gate tiles + linear head)
KC = 8                      # full K chunks of 128
F16 = mybir.dt.float16
F32 = mybir.dt.float32
NPF16 = np.float16

_CACHE: dict = {}


def _build_A(W_ih, W_hh, b_ih, b_hh, W_lin, b_lin):
    """(4224, 1028) fused matrix, fp32."""
    v = W_ih[:, 0].astype(np.float64)
    u = W_ih[:, 1].astype(np.float64)
    W_eff = W_hh.astype(np.float64) + np.outer(v, W_lin[0].astype(np.float64))
    bias_eff = (b_ih + b_hh).astype(np.float64) + u + float(b_lin[0]) * v

    A = np.zeros((4224, 1028), np.float64)
    gate_rows = {0: 0, 1: 1024, 2: 3072, 3: 2048}  # q -> i,f,o,g row base
    for j in range(8):
        for q in range(4):
            rows = slice(gate_rows[q] + 128 * j, gate_rows[q] + 128 * j + 128)
            m = 4 * j + q
            A[128 * m:128 * m + 128, 0:1024] = W_eff[rows]
            A[128 * m:128 * m + 128, 1024] = v[rows]
            A[128 * m:128 * m + 128, 1025] = u[rows]
            A[128 * m:128 * m + 128, 1026] = bias_eff[rows]
    A[4096, 0:1024] = W_lin[0]
    A[4096, 1026] = float(b_lin[0])
    return A.astype(np.float32)


def _trace_program():
    """Build + trace the Bass program (input-independent). Cached."""
    if "nc" in _CACHE:
        return _CACHE["nc"]

    nc = bacc.Bacc("TRN2", target_bir_lowering=False, debug=False,
                   num_devices=N_CORES)

    at_main = nc.dram_tensor("at_main", [128, KC, MT, 128], F16,
                             kind="ExternalInput")
    at_k8 = nc.dram_tensor("at_k8", [4, MT, 128], F16, kind="ExternalInput")
    h0t = nc.dram_tensor("h0t", [128, KC, BC], F16, kind="ExternalInput")
    c0t = nc.dram_tensor("c0t", [128, KC, BC], F32, kind="ExternalInput")
    k8v = nc.dram_tensor("k8v", [4, 2, BC], F16, kind="ExternalInput")
    brep = nc.dram_tensor("brep", [128, KC, 4, BC], F32, kind="ExternalInput")
    blin = nc.dram_tensor("blin", [1, 1], F32, kind="ExternalInput")
    y = nc.dram_tensor("y", [1, T * BC], F32, kind="ExternalOutput")

    MULT = mybir.AluOpType.mult
    SIG = mybir.ActivationFunctionType.Sigmoid
    TANH = mybir.ActivationFunctionType.Tanh
    IDENT = mybir.ActivationFunctionType.Identity

    with tile.TileContext(nc) as tc:
        with (
            tc.tile_pool(name="weights", bufs=1) as wpool,
            tc.tile_pool(name="state", bufs=1) as cpool,
            tc.tile_pool(name="hbuf", bufs=2) as hpool,
            tc.tile_pool(name="pslo", bufs=2, space="PSUM") as pslo_pool,
            tc.tile_pool(name="pshi", bufs=2, space="PSUM") as pshi_pool,
            tc.tile_pool(name="pso", bufs=2, space="PSUM") as pso_pool,
            tc.tile_pool(name="ew", bufs=3) as spool,
        ):
            atm = wpool.tile([128, KC, MT, 128], F16, tag="atm")
            nc.sync.dma_start(atm[:], at_main.ap())
            atk8 = wpool.tile([4, MT, 128], F16, tag="atk8")
            nc.sync.dma_start(atk8[:], at_k8.ap())
            k8t = wpool.tile([4, 2, BC], F16, tag="k8t")
            nc.sync.dma_start(k8t[:], k8v.ap())

            ct = cpool.tile([128, KC, BC], F32, tag="ct")
            nc.sync.dma_start(ct[:], c0t.ap())
            outs = cpool.tile([1, (T + 1) * BC], F32, tag="outs")
            brt = cpool.tile([128, KC, 4, BC], F32, tag="brt")
            nc.sync.dma_start(brt[:], brep.ap())
            blt = cpool.tile([1, 1], F32, tag="blt")
            nc.sync.dma_start(blt[:], blin.ap())

            h_prev = hpool.tile([128, KC, BC], F16, tag="h")
            nc.sync.dma_start(h_prev[:], h0t.ap())

            def tile32(po, h_rhs, k8rhs):
                # linear-head row: out(t-1) = W_lin h(t-1)  (+b_lin on copy)
                if k8rhs is not None:
                    nc.tensor.matmul(po[0:1, :], atk8[:, 32, 0:1], k8rhs,
                                     start=True, stop=False)
                for k in range(KC):
                    nc.tensor.matmul(po[0:1, :], atm[:, k, 32, 0:1],
                                     h_rhs[:, k, :],
                                     start=(k8rhs is None and k == 0),
                                     stop=(k == KC - 1))

            def copy_out(t, po):
                nc.scalar.activation(outs[0:1, BC * t:BC * (t + 1)],
                                     po[0:1, :], IDENT, bias=blt[0:1, 0:1])

            def emit_half(ps, jr, h_new):
                # LSTM cell for 4 h-chunks: ps is (128, 4, 4, 16) = [j,q,b]
                # q: 0=i 1=f 2=o 3=g.  Fused bias lives in brt, not PSUM.
                gs = spool.tile([128, 4, 4, BC], F32, tag="gs")
                nc.vector.tensor_add(gs[:], ps[:, :, :, :], brt[:, jr, :, :])
                sig = spool.tile([128, 4, 3, BC], F32, tag="sig")
                nc.scalar.activation(sig[:], gs[:, :, 0:3, :], SIG)
                tg = spool.tile([128, 4, BC], F32, tag="tg")
                nc.scalar.activation(tg[:], gs[:, :, 3, :], TANH)
                t1 = spool.tile([128, 4, BC], F32, tag="t1")
                nc.vector.tensor_tensor(t1[:], sig[:, :, 0, :], tg[:], MULT)
                t2 = spool.tile([128, 4, BC], F32, tag="t2")
                nc.vector.tensor_tensor(t2[:], sig[:, :, 1, :], ct[:, jr, :],
                                        MULT)
                nc.vector.tensor_add(ct[:, jr, :], t1[:], t2[:])
                tc_ = spool.tile([128, 4, BC], F32, tag="tc")
                nc.scalar.activation(tc_[:], ct[:, jr, :], TANH)
                nc.vector.tensor_tensor(h_new[:, jr, :], sig[:, :, 2, :],
                                        tc_[:], MULT)

            for t in range(T):
                # step-0 correction rows ride the k8 chunk; bias itself is
                # added by the DVE (brt), so k8 is skipped for t>=1.
                k8rhs = k8t[:, 0, :] if t == 0 else None
                ps_lo = pslo_pool.tile([128, 4, 4, BC], F32, tag="pslo")
                ps_hi = pshi_pool.tile([128, 4, 4, BC], F32, tag="pshi")
                po = pso_pool.tile([1, BC], F32, tag="po")
                h_new = hpool.tile([128, KC, BC], F16, tag="h")

                for m in range(32):
                    j, q = divmod(m, 4)
                    ps = ps_lo if j < 4 else ps_hi
                    out_ap = ps[:, j % 4, q, :]
                    if k8rhs is not None:
                        nc.tensor.matmul(out_ap, atk8[:, m, :], k8rhs,
                                         start=True, stop=False)
                    for k in range(KC):
                        nc.tensor.matmul(out_ap, atm[:, k, m, :],
                                         h_prev[:, k, :],
                                         start=(k8rhs is None and k == 0),
                                         stop=(k == KC - 1))
                    if m == 15:
                        emit_half(ps_lo, slice(0, 4), h_new)
                tile32(po, h_prev, k8rhs)
                copy_out(t, po)
                emit_half(ps_hi, slice(4, 8), h_new)
                h_prev = h_new

            # final linear head evaluation: out(T-1) from h(T-1)
            po = pso_pool.tile([1, BC], F32, tag="po")
            tile32(po, h_prev, None)
            copy_out(T, po)

            nc.sync.dma_start(y.ap(), outs[0:1, BC:])

    nc.compile()
    _CACHE["nc"] = nc
    return nc


def _prep_inputs(hidden_0, cell_0, W_ih, W_hh, b_ih, b_hh, W_lin, b_lin):
    A = _build_A(W_ih, W_hh, b_ih, b_hh, W_lin, b_lin)
    A16 = A.astype(NPF16)

    # stationary layouts: atm[p, k, m, c] = A[128m + c, 128k + p]
    Acore = A16[:4224, :1024].reshape(MT, 128, KC, 128)  # [m, c, k, p]
    atm = np.ascontiguousarray(Acore.transpose(3, 2, 0, 1))  # (128, KC, MT, 128)
    atk8 = np.ascontiguousarray(
        A16[:, 1024:1028].reshape(MT, 128, 4).transpose(2, 0, 1))  # (4, MT, 128)

    h0 = np.asarray(hidden_0, np.float32)[0]  # (B, H)
    c0 = np.asarray(cell_0, np.float32)[0]
    e0 = -(h0 @ np.asarray(W_lin, np.float32)[0] + float(np.asarray(b_lin)[0]))

    # fused bias replicated over batch: brep[p, j, q, b] = A[128*(4j+q)+p, 1026]
    bias_cols = A[:4096, 1026].reshape(32, 128)  # [m, p]
    brep = np.ascontiguousarray(
        np.broadcast_to(bias_cols.reshape(8, 4, 128)
                        .transpose(2, 0, 1)[:, :, :, None],
                        (128, KC, 4, BC))).astype(np.float32)
    blin = np.asarray(b_lin, np.float32).reshape(1, 1)

    in_maps = []
    for c in range(N_CORES):
        s = slice(BC * c, BC * (c + 1))
        h0t = np.ascontiguousarray(
            h0[s].T.reshape(KC, 128, BC).transpose(1, 0, 2)).astype(NPF16)
        c0t = np.ascontiguousarray(
            c0[s].T.reshape(KC, 128, BC).transpose(1, 0, 2)).astype(np.float32)
        k8v = np.zeros((4, 2, BC), NPF16)
        k8v[0, 0] = e0[s].astype(NPF16)
        k8v[1, 0] = -1.0
        in_maps.append({
            "at_main": atm, "at_k8": atk8,
            "h0t": h0t, "c0t": c0t, "k8v": k8v,
            "brep": brep, "blin": blin,
        })
    return in_maps


def kernel(hidden_0, cell_0, W_ih, W_hh, b_ih, b_hh, W_lin, b_lin,
           output_seq_len):
    assert int(output_seq_len) == T, f"compiled for T={T}"
    nc = _trace_program()
    in_maps = _prep_inputs(hidden_0, cell_0, W_ih, W_hh, b_ih, b_hh,
                           W_lin, b_lin)

    trace = bool(int(os.environ.get("KERNEL_TRACE", "0")))
    res = bass_utils.run_bass_kernel_spmd(
        nc, in_maps, core_ids=list(range(N_CORES)), trace=trace)
    _CACHE["last_result"] = res

    out = np.zeros((B, T, 1), np.float32)
    for c in range(N_CORES):
        yc = res.results[c]["y"].reshape(T, BC)  # [t, b]
        out[BC * c:BC * (c + 1), :, 0] = yc.T
    return out



# revision 18
# speedup vs baseline: 1.0032x; 1.0032x over previous
"""Trainium2 Bass kernel for the autoregressive LSTM decoder (nn_Decoder).

Problem: single-layer LSTM (H=1024) decoder, B=128 examples, T=256 steps,
autoregressive scalar feedback through a linear head (OUT=1), extra flag
input. Sequential over T.

Strategy (data-parallel, zero communication):
  * Shard batch over 8 cores (16 examples/core); replicate weights.
  * Per step compute gates in TRANSPOSED layout (gate-dim on partitions,
    batch on free dim) so the LSTM cell elementwise runs on all 128 lanes:
        gates.T = A @ h_ext
    A is a fused matrix built on host:
      - W_eff = W_hh + outer(W_ih[:,0], W_lin): the scalar feedback
        out(t-1) = W_lin h(t-1) + b_lin is folded into the recurrent
        matrix (rank-1 update), eliminating the serial feedback path.
      - k8 columns carry step-0 corrections; fused bias is DVE-added.
      - a separate fp16 row carries W_lin: emits out(t-1) per step.
  * The kernel is LDWEIGHTS-bound (batch free dim is only 16): per step
    264 stationary 128x128 tiles stream through the PE array. Weights are
    held as fp8e4m3 scaled by 2^5 (exponent shift; avoids the subnormal
    zone) so Fast-Weight-Load reads 4 weights per 32-bit word (2x fp16).
    The 2^-5 descale rides the ACT sigmoid/tanh `scale` operand for free.
  * Per-step schedule hides the cell-update (ACT+DVE) chain behind PE:
    matmuls run in k-pair waves consuming h chunks 0..7 in production
    order, so the tail half of step t's cell update overlaps the head
    waves of step t+1. h is split into lo/hi tiles for precise deps.
  * h state fp16 (unscaled), cell state c and accumulation fp32.
"""

import os
import numpy as np
import ml_dtypes

import concourse.bass as bass
import concourse.tile as tile
import concourse.bass_utils as bass_utils
from concourse import bacc, mybir

H = 1024
B = 128
T = 256
N_CORES = 8
BC = B // N_CORES          # batch per core = 16
KC = 8                      # full K chunks of 128
SCALE = float(os.environ.get("KERNEL_SCALE", "32.0"))  # weight exponent shift
SINV = 1.0 / SCALE
_W8 = bool(int(os.environ.get("KERNEL_W8", "1")))  # dev knob: fp8 weights
_WAVE = bool(int(os.environ.get("KERNEL_WAVE", "1")))  # dev knob: k-major waves
F8 = mybir.dt.float8e4 if _W8 else mybir.dt.float16
F16 = mybir.dt.float16
F32 = mybir.dt.float32
NPF8 = ml_dtypes.float8_e4m3 if _W8 else np.float16
NPF16 = np.float16

_CACHE: dict = {}


def _build_A(W_ih, W_hh, b_ih, b_hh, W_lin, b_lin):
    """(4224, 1028) fused matrix, fp32. Rows 0..4095: gate tiles in
    (j-chunk, q-gate) interleaved order; row 4096: W_lin (rest pad)."""
    v = W_ih[:, 0].astype(np.float64)
    u = W_ih[:, 1].astype(np.float64)
    W_eff = W_hh.astype(np.float64) + np.outer(v, W_lin[0].astype(np.float64))
    bias_eff = (b_ih + b_hh).astype(np.float64) + u + float(b_lin[0]) * v

    A = np.zeros((4224, 1028), np.float64)
    gate_rows = {0: 0, 1: 1024, 2: 3072, 3: 2048}  # q -> i,f,o,g row base
    for j in range(8):
        for q in range(4):
            rows = slice(gate_rows[q] + 128 * j, gate_rows[q] + 128 * j + 128)
            m = 4 * j + q
            A[128 * m:128 * m + 128, 0:1024] = W_eff[rows]
            A[128 * m:128 * m + 128, 1024] = v[rows]
            A[128 * m:128 * m + 128, 1025] = u[rows]
            A[128 * m:128 * m + 128, 1026] = bias_eff[rows]
    A[4096, 0:1024] = W_lin[0]
    A[4096, 1026] = float(b_lin[0])
    return A.astype(np.float32)


def _trace_program():
    """Build + trace the Bass program (input-independent). Cached."""
    if "nc" in _CACHE:
        return _CACHE["nc"]

    nc = bacc.Bacc("TRN2", target_bir_lowering=False, debug=False,
                   num_devices=N_CORES)

    at_main = nc.dram_tensor("at_main", [128, KC, 32, 128], F8,
                             kind="ExternalInput")
    at_lin = nc.dram_tensor("at_lin", [128, KC, 1], F16, kind="ExternalInput")
    at_k8 = nc.dram_tensor("at_k8", [4, 33, 128], F16, kind="ExternalInput")
    h0lo = nc.dram_tensor("h0lo", [128, 4, BC], F16, kind="ExternalInput")
    h0hi = nc.dram_tensor("h0hi", [128, 4, BC], F16, kind="ExternalInput")
    c0t = nc.dram_tensor("c0t", [128, KC, BC], F32, kind="ExternalInput")
    k8v = nc.dram_tensor("k8v", [4, 2, BC], F16, kind="ExternalInput")
    brep = nc.dram_tensor("brep", [128, KC, 4, BC], F32, kind="ExternalInput")
    blin = nc.dram_tensor("blin", [1, 1], F32, kind="ExternalInput")
    y = nc.dram_tensor("y", [1, T * BC], F32, kind="ExternalOutput")

    MULT = mybir.AluOpType.mult
    SIG = mybir.ActivationFunctionType.Sigmoid
    TANH = mybir.ActivationFunctionType.Tanh
    IDENT = mybir.ActivationFunctionType.Identity

    with tile.TileContext(nc) as tc:
        with (
            tc.tile_pool(name="weights", bufs=1) as wpool,
            tc.tile_pool(name="state", bufs=1) as cpool,
            tc.tile_pool(name="hlo", bufs=2) as hlo_pool,
            tc.tile_pool(name="hhi", bufs=2) as hhi_pool,
            tc.tile_pool(name="psloa", bufs=1, space="PSUM") as psloa_pool,
            tc.tile_pool(name="pslob", bufs=1, space="PSUM") as pslob_pool,
            tc.tile_pool(name="pshia", bufs=2, space="PSUM") as pshia_pool,
            tc.tile_pool(name="pshib", bufs=2, space="PSUM") as pshib_pool,
            tc.tile_pool(name="ew", bufs=3) as spool,
        ):
            atm = wpool.tile([128, KC, 32, 128], F8, tag="atm")
            nc.sync.dma_start(atm[:], at_main.ap())
            atl = wpool.tile([128, KC, 1], F16, tag="atl")
            nc.sync.dma_start(atl[:], at_lin.ap())
            atk8 = wpool.tile([4, 33, 128], F16, tag="atk8")
            nc.sync.dma_start(atk8[:], at_k8.ap())
            k8t = wpool.tile([4, 2, BC], F16, tag="k8t")
            nc.sync.dma_start(k8t[:], k8v.ap())

            ct = cpool.tile([128, KC, BC], F32, tag="ct")
            nc.sync.dma_start(ct[:], c0t.ap())
            outs = cpool.tile([1, (T + 1) * BC], F32, tag="outs")
            brt = cpool.tile([128, KC, 4, BC], F32, tag="brt")
            nc.sync.dma_start(brt[:], brep.ap())
            blt = cpool.tile([1, 1], F32, tag="blt")
            nc.sync.dma_start(blt[:], blin.ap())

            h_lo = hlo_pool.tile([128, 4, BC], F16, tag="hlo")
            nc.sync.dma_start(h_lo[:], h0lo.ap())
            h_hi = hhi_pool.tile([128, 4, BC], F16, tag="hhi")
            nc.sync.dma_start(h_hi[:], h0hi.ap())

            def hrhs(k):
                return h_lo[:, k, :] if k < 4 else h_hi[:, k - 4, :]

            def copy_out(t, po):
                nc.scalar.activation(outs[0:1, BC * t:BC * (t + 1)],
                                     po[0:1, :], IDENT, bias=blt[0:1, 0:1])

            def emit_half(psA, psB, jbase, h_new, stagger):
                # LSTM cell for 4 h-chunks: psA/psB are (128, 4, 4, 16) =
                # [j,q,b] partial gate sums over k=0..3 / k=4..7.
                # q: 0=i 1=f 2=o 3=g. Gates are in the x32 scaled domain;
                # the 2^-5 descale rides the ACT scale operand (fp32 affine).
                jr = slice(jbase, jbase + 4)
                # DVE reads at most one PSUM operand per instruction: fold
                # the bias in with half A, then add half B.
                gp = spool.tile([128, 4, 4, BC], F32, tag="gp")
                nc.vector.tensor_add(gp[:], psA[:, :, :, :], brt[:, jr, :, :])
                gs = spool.tile([128, 4, 4, BC], F32, tag="gs")
                nc.vector.tensor_add(gs[:], psB[:, :, :, :], gp[:])
                sig = spool.tile([128, 4, 3, BC], F32, tag="sig")
                nc.scalar.activation(sig[:], gs[:, :, 0:3, :], SIG, scale=SINV)
                tg = spool.tile([128, 4, BC], F32, tag="tg")
                nc.scalar.activation(tg[:], gs[:, :, 3, :], TANH, scale=SINV)
                t1 = spool.tile([128, 4, BC], F32, tag="t1")
                nc.vector.tensor_tensor(t1[:], sig[:, :, 0, :], tg[:], MULT)
                t2 = spool.tile([128, 4, BC], F32, tag="t2")
                nc.vector.tensor_tensor(t2[:], sig[:, :, 1, :], ct[:, jr, :],
                                        MULT)
                nc.vector.tensor_add(ct[:, jr, :], t1[:], t2[:])
                parts = ((0, 2), (2, 4)) if stagger else ((0, 4),)
                for (a, b) in parts:
                    w = b - a
                    jp = slice(jbase + a, jbase + b)
                    tc_ = spool.tile([128, w, BC], F32, tag="tc")
                    nc.scalar.activation(tc_[:], ct[:, jp, :], TANH)
                    nc.vector.tensor_tensor(h_new[:, a:b, :],
                                            sig[:, a:b, 2, :], tc_[:], MULT)

            for t in range(T):
                ps_loa = psloa_pool.tile([128, 4, 4, BC], F32, tag="psloa")
                ps_lob = pslob_pool.tile([128, 4, 4, BC], F32, tag="pslob")
                ps_hia = pshia_pool.tile([128, 4, 4, BC], F32, tag="pshia")
                ps_hib = pshib_pool.tile([128, 4, 4, BC], F32, tag="pshib")
                # po shares pslob's bank; its start=True fires only after
                # every LO_B group has finished accumulating, so the bank-
                # wide has_written clear cannot corrupt them.
                po = pslob_pool.tile([1, BC], F32, tag="po")
                h_lo_new = hlo_pool.tile([128, 4, BC], F16, tag="hlo")
                h_hi_new = hhi_pool.tile([128, 4, BC], F16, tag="hhi")

                def psv(m, half):
                    if half == 0:
                        ps = ps_loa if m < 16 else ps_hia
                    else:
                        ps = ps_lob if m < 16 else ps_hib
                    j, q = divmod(m % 16, 4)
                    return ps[:, j, q, :]

                k8rhs = k8t[:, 0, :] if t == 0 else None

                def gate_tile(m, half):
                    # one contiguous 4-chunk accumulation group per PSUM
                    # region; step-0 corrections ride the k8 rows (half A).
                    ks = range(4) if half == 0 else range(4, KC)
                    first = half == 1 or k8rhs is None
                    if half == 0 and k8rhs is not None:
                        nc.tensor.matmul(psv(m, 0), atk8[:, m, :], k8rhs,
                                         start=True, stop=False)
                    for i, k in enumerate(ks):
                        nc.tensor.matmul(psv(m, half), atm[:, k, m, :],
                                         hrhs(k), start=(first and i == 0),
                                         stop=(i == 3))

                # Phase order defers late-chunk (4..7) consumption so the
                # previous step's hi-half cell update hides under ~2x the
                # matmul runway of phases LO_A + HI_A.
                for m in range(16):          # LO_A: chunks 0..3
                    gate_tile(m, 0)
                for m in range(16, 32):      # HI_A: chunks 0..3
                    gate_tile(m, 0)
                for m in range(16):          # LO_B: chunks 4..7
                    gate_tile(m, 1)
                # linear head row: contiguous k=0..7 group (all chunks are
                # available by now); k8 start only at t=0.
                if k8rhs is not None:
                    nc.tensor.matmul(po[0:1, :], atk8[:, 32, 0:1], k8rhs,
                                     start=True, stop=False)
                for k in range(KC):
                    nc.tensor.matmul(po[0:1, :], atl[:, k, 0:1], hrhs(k),
                                     start=(k8rhs is None and k == 0),
                                     stop=(k == KC - 1))

                copy_out(t, po)
                emit_half(ps_loa, ps_lob, 0, h_lo_new, stagger=False)

                for m in range(16, 32):      # HI_B: overlaps emit of lo half
                    gate_tile(m, 1)

                emit_half(ps_hia, ps_hib, 4, h_hi_new, stagger=True)
                h_lo, h_hi = h_lo_new, h_hi_new

            # final linear head evaluation: out(T-1) from h(T-1)
            po = pslob_pool.tile([1, BC], F32, tag="po")
            for k in range(KC):
                nc.tensor.matmul(po[0:1, :], atl[:, k, 0:1], hrhs(k),
                                 start=(k == 0), stop=(k == KC - 1))
            copy_out(T, po)

            nc.sync.dma_start(y.ap(), outs[0:1, BC:])

    nc.compile()
    _CACHE["nc"] = nc
    return nc


def _prep_inputs(hidden_0, cell_0, W_ih, W_hh, b_ih, b_hh, W_lin, b_lin):
    A = _build_A(W_ih, W_hh, b_ih, b_hh, W_lin, b_lin)

    # gate tiles, x32 exponent shift, fp8: atm[p, k, m, c] = A[128m+c, 128k+p]
    Acore = (A[:4096, :1024] * SCALE).astype(NPF8)
    Acore = Acore.reshape(32, 128, KC, 128)              # [m, c, k, p]
    atm = np.ascontiguousarray(Acore.transpose(3, 2, 0, 1))  # (128, KC, 32, 128)

    # linear head row kept fp16 unscaled: y precision is not fp8-limited
    at_lin = np.ascontiguousarray(
        A[4096, :1024].reshape(KC, 128).transpose(1, 0)[:, :, None]
    ).astype(NPF16)                                      # (128, KC, 1)

    # k8 correction columns: x32 for the gate tiles (PSUM domain is x32)
    atk8_f = A[:, 1024:1028].reshape(33, 128, 4).transpose(2, 0, 1).copy()
    atk8_f[:, :32, :] *= SCALE
    atk8 = atk8_f.astype(NPF16)                          # (4, 33, 128)

    h0 = np.asarray(hidden_0, np.float32)[0]  # (B, H)
    c0 = np.asarray(cell_0, np.float32)[0]
    e0 = -(h0 @ np.asarray(W_lin, np.float32)[0] + float(np.asarray(b_lin)[0]))

    # fused bias (x32) replicated over batch: brep[p, j, q, b]
    bias_cols = A[:4096, 1026].reshape(32, 128) * SCALE  # [m, p]
    brep = np.ascontiguousarray(
        np.broadcast_to(bias_cols.reshape(8, 4, 128)
                        .transpose(2, 0, 1)[:, :, :, None],
                        (128, KC, 4, BC))).astype(np.float32)
    blin = np.asarray(b_lin, np.float32).reshape(1, 1)

    in_maps = []
    for c in range(N_CORES):
        s = slice(BC * c, BC * (c + 1))
        h0t = np.ascontiguousarray(
            h0[s].T.reshape(KC, 128, BC).transpose(1, 0, 2)).astype(NPF16)
        c0t = np.ascontiguousarray(
            c0[s].T.reshape(KC, 128, BC).transpose(1, 0, 2)).astype(np.float32)
        k8vv = np.zeros((4, 2, BC), NPF16)
        k8vv[0, 0] = e0[s].astype(NPF16)
        k8vv[1, 0] = -1.0
        in_maps.append({
            "at_main": atm, "at_lin": at_lin, "at_k8": atk8,
            "h0lo": np.ascontiguousarray(h0t[:, 0:4, :]),
            "h0hi": np.ascontiguousarray(h0t[:, 4:8, :]),
            "c0t": c0t, "k8v": k8vv,
            "brep": brep, "blin": blin,
        })
    return in_maps


def kernel(hidden_0, cell_0, W_ih, W_hh, b_ih, b_hh, W_lin, b_lin,
           output_seq_len):
    assert int(output_seq_len) == T, f"compiled for T={T}"
    nc = _trace_program()
    in_maps = _prep_inputs(hidden_0, cell_0, W_ih, W_hh, b_ih, b_hh,
                           W_lin, b_lin)

    trace = bool(int(os.environ.get("KERNEL_TRACE", "0")))
    res = bass_utils.run_bass_kernel_spmd(
        nc, in_maps, core_ids=list(range(N_CORES)), trace=trace)
    _CACHE["last_result"] = res

    out = np.zeros((B, T, 1), np.float32)
    for c in range(N_CORES):
        yc = res.results[c]["y"].reshape(T, BC)  # [t, b]
        out[BC * c:BC * (c + 1), :, 0] = yc.T
    return out
